# revision 43
# baseline (speedup 1.0000x reference)
"""ConvTranspose3d(64->32, k=3, stride=2, pad=1, out_pad=1, dilation=2) on 8 NeuronCores.

Math: with stride=2, dilation=2, padding=1, k=3, the transposed conv collapses
to a dense 3^3 conv y = conv3d(x, wc, padding=1) on the 32^3 grid
(wc = flip(transpose(w))), scattered into the odd sub-lattice of the 66^3
output; every other output voxel is just bias (added host-side).

Sharding: 8 shards = 2 batches x 2 depth-halves x 2 h-halves; each core owns
16 conv output planes x 16 h rows x 32 w.

Kernel: sliding-pair block-Toeplitz implicit GEMM. K = (64 c_in x 2 adjacent
padded input planes) per pair p (9 pairs cover the 18-plane slab), M = (4
output planes {2p-2..2p+1} x 32 c_out, column = (q mod 4)*32+co), N = 512 hw
px. Each pair accumulates its 9 hw taps into its own PSUM bank (81 matmuls
vs 108 for the quad-aligned 50%-dense tiling); every output plane q is the
sum of two banks (floor(q/2), floor(q/2)+1) at the same partitions. The odd
bank of each combine is copied PSUM->SBUF on the scalar engine (frees its
bank early; the ISA allows only one PSUM operand per DVE op), then DVE
tensor_add(SBUF, PSUM) -> bf16 SBUF -> DMA out. Bias is added host-side
(every non-conv voxel is exactly bias, so the device ships conv voxels
only, 0.5MB bf16 per core).

Edge pairs 0 and 8 are half-useful (planes {0,1} in columns 0:64 and
{14,15} in 64:128 of the same parity-0 stationary), so they run last as
column-tiled concurrent matmul pairs (tile_position (0,0)/(0,64)): 18
matmuls in ~11 spans. Pair 8 leads by two taps so its combine retires
inside the stream and only c_0 sits on the critical tail.

Only two distinct stationary patterns exist (pair parity), so tcw ships as
2x9x[128,128] bf16 (590KB). Perf notes from perfetto: ~7us fixed NEFF
preamble, first DMA flow ~8.2us, ~0.9us DMA-completion semaphore latency,
back-to-back N=512 bf16 matmuls at ~218ns (2.4GHz; chip sometimes sits at
~2.0GHz P0). Inputs ship in first-use order with per-pair semaphores on
three queues sized so stationary delivery (~150KB/us consumption) keeps
ahead of the PE; warmup matmuls on memset garbage during the input window
ramp the PE HAM clock gate (1.2->2.4GHz after ~3.4us of busy).
"""

import sys

sys.path.insert(0, "/opt/trn_rl_repo")

import numpy as np

N_CORES = 8
N_PAIRS = 9  # input plane pairs per core (18 padded planes)

# [parity][dpi][qblk] = kd of the weight block, omitted -> structural zero
_KD = {
    0: {0: {0: 0, 2: 2, 3: 1}, 1: {0: 1, 1: 0, 3: 2}},
    1: {0: {0: 2, 1: 1, 2: 0}, 1: {1: 2, 2: 1, 3: 0}},
}

_cache = {}


def _build_nc():
    import concourse.bass as bass  # noqa: F401
    import concourse.tile as tile
    from concourse import bacc, mybir

    dt = mybir.dt
    nc = bacc.Bacc("TRN2", target_bir_lowering=False, debug=False,
                   num_devices=N_CORES)

    # xs: 9 pairs of adjacent padded input planes; partition p = dpi*64 + ci,
    # free (pair, 18 h rows, 34 w). tcw: 2 parity patterns x 9 hw taps of
    # [128, 128] block-Toeplitz stationary, columns (q mod 4)*32 + co.
    xs = nc.dram_tensor("xs", [128, N_PAIRS, 18, 34], dt.bfloat16,
                        kind="ExternalInput")
    tcw = nc.dram_tensor("tcw", [128, 2, 9, 128], dt.bfloat16,
                         kind="ExternalInput")
    # conv voxels only: partition (qblk*32+co), j in 0..3, plane q = 4j+qblk,
    # px = (16h, 32w); bias is added host-side.
    out = nc.dram_tensor("out", [128, 4, 512], dt.bfloat16,
                         kind="ExternalOutput")

    with tile.TileContext(nc) as tc:
        with (
            tc.tile_pool(name="tw", bufs=1) as tw_pool,
            tc.tile_pool(name="xp", bufs=1) as xp_pool,
            tc.tile_pool(name="ot", bufs=4) as ot_pool,
            tc.tile_pool(name="dm", bufs=1) as dm_pool,
            tc.tile_pool(name="ps", bufs=8, space="PSUM") as ps_pool,
        ):
            tw_t = tw_pool.tile([128, 2, 9, 128], dt.bfloat16)
            xs_t = xp_pool.tile([128, N_PAIRS, 18, 34], dt.bfloat16)
            ot = [ot_pool.tile([128, 512], dt.bfloat16, tag=f"ot{j}",
                               name=f"ot{j}")
                  for j in range(4)]
            # SBUF copies of the odd PSUM banks (DVE can't read 2 PSUM srcs)
            sbc = [ot_pool.tile([128, 512], dt.float32, tag=f"sbc{i}",
                                name=f"sbc{i}")
                   for i in range(4)]
            dummy = dm_pool.tile([128, 512], dt.bfloat16)

            # memset on gpsimd: its queue opens ~1.4us before vector's, so
            # the warmup matmuls (and the HAM clock ramp) start that much
            # earlier
            nc.gpsimd.memset(dummy[:], 0.0)

            # DMAs in exact first-use order, critical prefix smallest: the
            # first matmul needs only tw[pi0, taps 0:3] + xs pair 0. Sync
            # carries the stationary, scalar the x slab; gpsimd/vector are
            # kept free for the output stores.
            # pair order is 1..7 then {0,8} merged, so pi=1 taps ship first.
            # pair 1 rides gpsimd's otherwise-idle queue so the critical
            # first chunk doesn't share a DMA ramp with the stationary.
            nc.sync.dma_start(tw_t[:, 1, 0:5], tcw[:, 1, 0:5])
            nc.sync.dma_start(tw_t[:, 1, 5:9], tcw[:, 1, 5:9])
            nc.gpsimd.dma_start(tw_t[:, 0, 0:5], tcw[:, 0, 0:5])
            nc.gpsimd.dma_start(tw_t[:, 0, 5:9], tcw[:, 0, 5:9])
            nc.scalar.dma_start(xs_t[:, 1:2], xs[:, 1:2])
            nc.scalar.dma_start(xs_t[:, 2:3], xs[:, 2:3])
            nc.scalar.dma_start(xs_t[:, 3:5], xs[:, 3:5])
            nc.scalar.dma_start(xs_t[:, 5:8], xs[:, 5:8])
            nc.gpsimd.dma_start(xs_t[:, 8:9], xs[:, 8:9])
            nc.gpsimd.dma_start(xs_t[:, 0:1], xs[:, 0:1])

            # warmup matmuls on zeroed garbage ramp the PE clock (HAM) during
            # the input-DMA window so the real matmuls run at full speed;
            # they bridge the PE-busy window until pair 0 lands so the HAM
            # ramp isn't reset by an idle gap
            wps = ps_pool.tile([128, 512], dt.float32, name="psb")
            warm_mm = None
            for _ in range(4):
                warm_mm = nc.tensor.matmul(wps[:], dummy[:, 0:128], dummy[:],
                                           start=True, stop=True)

            ps = {}
            state = {"prev": warm_mm}

            def chain(mm):
                # keep the PE static order pair-contiguous so banks retire
                # in order and the PE never splits an accumulation
                if state["prev"] is not None:
                    tile.add_dep_helper(mm.ins, state["prev"].ins, sync=False,
                                        reason="pair-contiguous PE order")

            # combine c: planes {2c, 2c+1} = even bank + odd-bank SBUF copy
            # at partitions (c%2)*64..+64, written into out tile j = c//2.
            # Stores are merged to full tiles where both halves retire
            # mid-stream (fewer DMAs -> fewer semaphores -> shorter
            # end-of-kernel semaphore-clear sweep).
            def combine(c, st_eng, store="full"):
                j, half = c // 2, c % 2
                sl = slice(64 * half, 64 * half + 64)
                if half == 0:
                    nc.vector.tensor_add(ot[j][sl, :], ps[c][sl, :],
                                         sbc[c // 2][sl, :])
                else:
                    nc.vector.tensor_add(ot[j][sl, :], sbc[c // 2][sl, :],
                                         ps[c + 1][sl, :])
                if store == "full":
                    st_eng.dma_start(out[:, j, :], ot[j][:])
                elif store == "half":
                    st_eng.dma_start(out[sl, j, :], ot[j][sl, :])

            # interior pairs 1..7 first (their banks all retire mid-stream);
            # the half-useful edge pairs 0 and 8 run last as column-tiled
            # concurrent matmul pairs (their x chunks also arrive last)
            for p in range(1, 8):
                pi = p % 2
                ps_p = ps_pool.tile([128, 512], dt.float32, name="psb")
                ps[p] = ps_p
                for t in range(9):
                    kh, kw = t // 3, t % 3
                    mm = nc.tensor.matmul(
                        ps_p[:], tw_t[:, pi, t],
                        xs_t[:, p, kh:kh + 16, kw:kw + 32],
                        start=(t == 0), stop=(t == 8))
                    if t == 0:
                        chain(mm)
                state["prev"] = mm

                # odd banks get a scalar-engine PSUM->SBUF copy (frees the
                # bank early; the combines then read SBUF + one PSUM)
                if p % 2 == 1:
                    nc.scalar.copy(sbc[p // 2][:], ps_p[:])
                if p >= 2:
                    c = p - 1
                    # j=0: half-store now (its other half is tail c_0);
                    # j=1,2: full store once the odd half completes;
                    # j=3 (c_6): deferred, stored whole after c_7
                    combine(c, nc.gpsimd,
                            store=("half" if c == 1 else
                                   "full" if c in (3, 5) else "none"))

            # pairs 0 and 8 share the pi=0 stationary: pair 0's planes {0,1}
            # live in columns 0:64, pair 8's planes {14,15} in 64:128, so
            # each tap runs as two concurrent column-tiled matmuls (pair 0
            # staggered one tap ahead so its combine starts a span earlier)
            ps[0] = ps_pool.tile([128, 512], dt.float32, name="psb")
            ps[8] = ps_pool.tile([128, 512], dt.float32, name="psb")

            def mm_edge(p, t, tp):
                kh, kw = t // 3, t % 3
                lo = 64 * (tp[1] // 64)
                return nc.tensor.matmul(
                    ps[p][lo:lo + 64, :], tw_t[:, 0, t, lo:lo + 64],
                    xs_t[:, p, kh:kh + 16, kw:kw + 32],
                    start=(t == 0), stop=(t == 8), tile_position=tp)

            # pair 8 leads by two taps so it finishes two spans early: its
            # combine c_7 (whose bank-7 partner retired long ago) then runs
            # inside the stream, leaving only c_0 on the critical tail
            mm = mm_edge(8, 0, (0, 64))
            chain(mm)
            mm_edge(8, 1, (0, 64))
            for t in range(2, 9):
                mm_edge(8, t, (0, 64))
                mm_edge(0, t - 2, (0, 0))
            mm_edge(0, 7, (0, 0))
            mm_edge(0, 8, (0, 0))

            combine(7, nc.scalar)  # c_6 half already written: one full store
            combine(0, nc.sync, store="half")

    nc.compile()
    return nc


def _prep_tcw(weight):
    import ml_dtypes

    # wc[co, ci, kd, kh, kw] = weight[ci, co, 2-kd, 2-kh, 2-kw]
    wc = np.flip(np.transpose(weight, (1, 0, 2, 3, 4)), axis=(2, 3, 4))
    tcw = np.zeros((128, 2, 9, 128), np.float32)
    for pi in range(2):
        for dpi in range(2):
            for qblk, kd in _KD[pi][dpi].items():
                # [ci, t, co]
                arr = wc[:, :, kd].reshape(32, 64, 9).transpose(1, 2, 0)
                tcw[dpi * 64:(dpi + 1) * 64, pi, :,
                    qblk * 32:(qblk + 1) * 32] = arr
    return np.ascontiguousarray(tcw.astype(ml_dtypes.bfloat16))


def _make_in_maps(x, weight, bias):
    import ml_dtypes

    tcwb = _prep_tcw(weight)
    in_maps = []
    for core in range(N_CORES):
        n, dh, hh = core // 4, (core // 2) % 2, core % 2
        xpad = np.pad(x[n], ((0, 0), (1, 1), (1, 1), (1, 1)))
        xsl = np.empty((128, N_PAIRS, 18, 34), ml_dtypes.bfloat16)
        for dpi in range(2):
            xsl[dpi * 64:(dpi + 1) * 64] = xpad[
                :, 16 * dh + dpi:16 * dh + dpi + 17:2,
                16 * hh:16 * hh + 18, :]
        in_maps.append({"xs": xsl, "tcw": tcwb})
    return in_maps


def kernel(x, weight, bias):
    from concourse.bass_utils import run_bass_kernel_spmd

    if "nc" not in _cache:
        _cache["nc"] = _build_nc()
    nc = _cache["nc"]

    x = np.asarray(x, np.float32)
    weight = np.asarray(weight, np.float32)
    bias = np.asarray(bias, np.float32)

    in_maps = _make_in_maps(x, weight, bias)
    res = run_bass_kernel_spmd(nc, in_maps, core_ids=list(range(N_CORES)))

    # every non-conv voxel (even lattice positions, trailing output_padding
    # planes) is exactly bias; fill host-side, add bias to the conv voxels
    # and scatter them into the odd sub-lattice
    full = np.empty((2, 32, 66, 66, 66), np.float32)
    full[:] = bias.reshape(1, 32, 1, 1, 1)
    for core in range(N_CORES):
        n, dh, hh = core // 4, (core // 2) % 2, core % 2
        # [128, 4, 512] -> (qblk, co, j, h, w) -> (co, q=4j+qblk, h, w)
        arr = res.results[core]["out"].astype(np.float32)
        arr = arr.reshape(4, 32, 4, 16, 32).transpose(1, 2, 0, 3, 4)
        arr = arr.reshape(32, 16, 16, 32) + bias.reshape(32, 1, 1, 1)
        full[n, :, 32 * dh + 1:32 * dh + 32:2,
             32 * hh + 1:32 * hh + 32:2, 1:65:2] = arr
    return full


# revision 44
# speedup vs baseline: 1.0449x; 1.0449x over previous
"""ConvTranspose3d(64->32, k=3, stride=2, pad=1, out_pad=1, dilation=2) on 8 NeuronCores.

Math: with stride=2, dilation=2, padding=1, k=3, the transposed conv collapses
to a dense 3^3 conv y = conv3d(x, wc, padding=1) on the 32^3 grid
(wc = flip(transpose(w))), scattered into the odd sub-lattice of the 66^3
output; every other output voxel is just bias (added host-side).

Sharding: 8 shards = 2 batches x 2 depth-halves x 2 h-halves; each core owns
16 conv output planes x 16 h rows x 32 w.

Kernel: sliding-pair block-Toeplitz implicit GEMM. K = (64 c_in x 2 adjacent
padded input planes) per pair p (9 pairs cover the 18-plane slab), M = (4
output planes {2p-2..2p+1} x 32 c_out, column = (q mod 4)*32+co), N = 512 hw
px. Each pair accumulates its 9 hw taps into its own PSUM bank (81 matmuls
vs 108 for the quad-aligned 50%-dense tiling); every output plane q is the
sum of two banks (floor(q/2), floor(q/2)+1) at the same partitions. The odd
bank of each combine is copied PSUM->SBUF on the scalar engine (frees its
bank early; the ISA allows only one PSUM operand per DVE op), then DVE
tensor_add(SBUF, PSUM) -> bf16 SBUF -> DMA out. Bias is added host-side
(every non-conv voxel is exactly bias, so the device ships conv voxels
only, 0.5MB bf16 per core).

Edge pairs 0 and 8 are half-useful (planes {0,1} in columns 0:64 and
{14,15} in 64:128 of the same parity-0 stationary), so they run last as
column-tiled concurrent matmul pairs (tile_position (0,0)/(0,64)): 18
matmuls in ~11 spans. Pair 8 leads by two taps so its combine retires
inside the stream and only c_0 sits on the critical tail.

Only two distinct stationary patterns exist (pair parity), so tcw ships as
2x9x[128,128] bf16 (590KB). Perf notes from perfetto: ~7us fixed NEFF
preamble, first DMA flow ~8.2us, ~0.9us DMA-completion semaphore latency,
back-to-back N=512 bf16 matmuls at ~218ns (2.4GHz; chip sometimes sits at
~2.0GHz P0). Inputs ship in first-use order with per-pair semaphores on
three queues sized so stationary delivery (~150KB/us consumption) keeps
ahead of the PE; warmup matmuls on memset garbage during the input window
ramp the PE HAM clock gate (1.2->2.4GHz after ~3.4us of busy).
"""

import sys

sys.path.insert(0, "/opt/trn_rl_repo")

import numpy as np

N_CORES = 8
N_PAIRS = 9  # input plane pairs per core (18 padded planes)

# [parity][dpi][qblk] = kd of the weight block, omitted -> structural zero
_KD = {
    0: {0: {0: 0, 2: 2, 3: 1}, 1: {0: 1, 1: 0, 3: 2}},
    1: {0: {0: 2, 1: 1, 2: 0}, 1: {1: 2, 2: 1, 3: 0}},
}

_cache = {}


def _build_nc():
    import concourse.bass as bass  # noqa: F401
    import concourse.tile as tile
    from concourse import bacc, mybir

    dt = mybir.dt
    nc = bacc.Bacc("TRN2", target_bir_lowering=False, debug=False,
                   num_devices=N_CORES)

    # xs: 9 pairs of adjacent padded input planes; partition p = dpi*64 + ci,
    # free (pair, 18 h rows, 34 w). tcw: 2 parity patterns x 9 hw taps of
    # [128, 128] block-Toeplitz stationary, columns (q mod 4)*32 + co.
    xs = nc.dram_tensor("xs", [128, N_PAIRS, 18, 34], dt.bfloat16,
                        kind="ExternalInput")
    tcw = nc.dram_tensor("tcw", [128, 2, 9, 128], dt.bfloat16,
                         kind="ExternalInput")
    # conv voxels only: partition (qblk*32+co), j in 0..3, plane q = 4j+qblk,
    # px = (16h, 32w); bias is added host-side.
    out = nc.dram_tensor("out", [128, 4, 512], dt.bfloat16,
                         kind="ExternalOutput")

    with tile.TileContext(nc) as tc:
        with (
            tc.tile_pool(name="tw", bufs=1) as tw_pool,
            tc.tile_pool(name="xp", bufs=1) as xp_pool,
            tc.tile_pool(name="ot", bufs=4) as ot_pool,
            tc.tile_pool(name="dm", bufs=1) as dm_pool,
            tc.tile_pool(name="ps", bufs=8, space="PSUM") as ps_pool,
        ):
            tw_t = tw_pool.tile([128, 2, 9, 128], dt.bfloat16)
            xs_t = xp_pool.tile([128, N_PAIRS, 18, 34], dt.bfloat16)
            ot = [ot_pool.tile([128, 512], dt.bfloat16, tag=f"ot{j}",
                               name=f"ot{j}")
                  for j in range(4)]
            # SBUF copies of the odd PSUM banks (DVE can't read 2 PSUM srcs)
            sbc = [ot_pool.tile([128, 512], dt.float32, tag=f"sbc{i}",
                                name=f"sbc{i}")
                   for i in range(4)]
            dummy = dm_pool.tile([128, 512], dt.bfloat16)

            # memset on gpsimd: its queue opens ~1.4us before vector's, so
            # the warmup matmuls (and the HAM clock ramp) start that much
            # earlier
            nc.gpsimd.memset(dummy[:], 0.0)

            # DMAs in exact first-use order, critical prefix smallest: the
            # first matmul needs only tw[pi0, taps 0:3] + xs pair 0. Sync
            # carries the stationary, scalar the x slab; gpsimd/vector are
            # kept free for the output stores.
            # pair order is 1..7 then {0,8} merged, so pi=1 taps ship first.
            # pair 1 rides gpsimd's otherwise-idle queue so the critical
            # first chunk doesn't share a DMA ramp with the stationary.
            nc.sync.dma_start(tw_t[:, 1, 0:5], tcw[:, 1, 0:5])
            nc.sync.dma_start(tw_t[:, 1, 5:9], tcw[:, 1, 5:9])
            nc.gpsimd.dma_start(tw_t[:, 0, 0:5], tcw[:, 0, 0:5])
            nc.gpsimd.dma_start(tw_t[:, 0, 5:9], tcw[:, 0, 5:9])
            nc.scalar.dma_start(xs_t[:, 1:2], xs[:, 1:2])
            nc.scalar.dma_start(xs_t[:, 2:3], xs[:, 2:3])
            nc.scalar.dma_start(xs_t[:, 3:5], xs[:, 3:5])
            nc.scalar.dma_start(xs_t[:, 5:8], xs[:, 5:8])
            nc.gpsimd.dma_start(xs_t[:, 8:9], xs[:, 8:9])
            nc.gpsimd.dma_start(xs_t[:, 0:1], xs[:, 0:1])

            # warmup matmuls on zeroed garbage ramp the PE clock (HAM) during
            # the input-DMA window so the real matmuls run at full speed;
            # they bridge the PE-busy window until pair 0 lands so the HAM
            # ramp isn't reset by an idle gap
            wps = ps_pool.tile([128, 512], dt.float32, name="psb")
            warm_mm = None
            for _ in range(5):
                warm_mm = nc.tensor.matmul(wps[:], dummy[:, 0:128], dummy[:],
                                           start=True, stop=True)

            ps = {}
            state = {"prev": warm_mm}

            def chain(mm):
                # keep the PE static order pair-contiguous so banks retire
                # in order and the PE never splits an accumulation
                if state["prev"] is not None:
                    tile.add_dep_helper(mm.ins, state["prev"].ins, sync=False,
                                        reason="pair-contiguous PE order")

            # combine c: planes {2c, 2c+1} = even bank + odd-bank SBUF copy
            # at partitions (c%2)*64..+64, written into out tile j = c//2.
            # Stores are merged to full tiles where both halves retire
            # mid-stream (fewer DMAs -> fewer semaphores -> shorter
            # end-of-kernel semaphore-clear sweep).
            def combine(c, st_eng, store="full"):
                j, half = c // 2, c % 2
                sl = slice(64 * half, 64 * half + 64)
                if half == 0:
                    nc.vector.tensor_add(ot[j][sl, :], ps[c][sl, :],
                                         sbc[c // 2][sl, :])
                else:
                    nc.vector.tensor_add(ot[j][sl, :], sbc[c // 2][sl, :],
                                         ps[c + 1][sl, :])
                if store == "full":
                    st_eng.dma_start(out[:, j, :], ot[j][:])
                elif store == "half":
                    st_eng.dma_start(out[sl, j, :], ot[j][sl, :])

            # interior pairs 1..7 first (their banks all retire mid-stream);
            # the half-useful edge pairs 0 and 8 run last as column-tiled
            # concurrent matmul pairs (their x chunks also arrive last)
            for p in range(1, 8):
                pi = p % 2
                ps_p = ps_pool.tile([128, 512], dt.float32, name="psb")
                ps[p] = ps_p
                for t in range(9):
                    kh, kw = t // 3, t % 3
                    mm = nc.tensor.matmul(
                        ps_p[:], tw_t[:, pi, t],
                        xs_t[:, p, kh:kh + 16, kw:kw + 32],
                        start=(t == 0), stop=(t == 8))
                    if t == 0:
                        chain(mm)
                state["prev"] = mm

                # odd banks get a scalar-engine PSUM->SBUF copy (frees the
                # bank early; the combines then read SBUF + one PSUM)
                if p % 2 == 1:
                    nc.scalar.copy(sbc[p // 2][:], ps_p[:])
                if p >= 2:
                    c = p - 1
                    # j=0: half-store now (its other half is tail c_0);
                    # j=1,2: full store once the odd half completes;
                    # j=3 (c_6): deferred, stored whole after c_7
                    combine(c, nc.gpsimd,
                            store=("half" if c == 1 else
                                   "full" if c in (3, 5) else "none"))

            # pairs 0 and 8 share the pi=0 stationary: pair 0's planes {0,1}
            # live in columns 0:64, pair 8's planes {14,15} in 64:128, so
            # each tap runs as two concurrent column-tiled matmuls (pair 0
            # staggered one tap ahead so its combine starts a span earlier)
            ps[0] = ps_pool.tile([128, 512], dt.float32, name="psb")
            ps[8] = ps_pool.tile([128, 512], dt.float32, name="psb")

            def mm_edge(p, t, tp):
                kh, kw = t // 3, t % 3
                lo = 64 * (tp[1] // 64)
                return nc.tensor.matmul(
                    ps[p][lo:lo + 64, :], tw_t[:, 0, t, lo:lo + 64],
                    xs_t[:, p, kh:kh + 16, kw:kw + 32],
                    start=(t == 0), stop=(t == 8), tile_position=tp)

            # pair 8 leads by two taps so it finishes two spans early: its
            # combine c_7 (whose bank-7 partner retired long ago) then runs
            # inside the stream, leaving only c_0 on the critical tail
            mm = mm_edge(8, 0, (0, 64))
            chain(mm)
            mm_edge(8, 1, (0, 64))
            for t in range(2, 9):
                mm_edge(8, t, (0, 64))
                mm_edge(0, t - 2, (0, 0))
            mm_edge(0, 7, (0, 0))
            mm_edge(0, 8, (0, 0))

            combine(7, nc.scalar)  # c_6 half already written: one full store
            combine(0, nc.sync, store="half")

    nc.compile()
    return nc


def _prep_tcw(weight):
    import ml_dtypes

    # wc[co, ci, kd, kh, kw] = weight[ci, co, 2-kd, 2-kh, 2-kw]
    wc = np.flip(np.transpose(weight, (1, 0, 2, 3, 4)), axis=(2, 3, 4))
    tcw = np.zeros((128, 2, 9, 128), np.float32)
    for pi in range(2):
        for dpi in range(2):
            for qblk, kd in _KD[pi][dpi].items():
                # [ci, t, co]
                arr = wc[:, :, kd].reshape(32, 64, 9).transpose(1, 2, 0)
                tcw[dpi * 64:(dpi + 1) * 64, pi, :,
                    qblk * 32:(qblk + 1) * 32] = arr
    return np.ascontiguousarray(tcw.astype(ml_dtypes.bfloat16))


def _make_in_maps(x, weight, bias):
    import ml_dtypes

    tcwb = _prep_tcw(weight)
    in_maps = []
    for core in range(N_CORES):
        n, dh, hh = core // 4, (core // 2) % 2, core % 2
        xpad = np.pad(x[n], ((0, 0), (1, 1), (1, 1), (1, 1)))
        xsl = np.empty((128, N_PAIRS, 18, 34), ml_dtypes.bfloat16)
        for dpi in range(2):
            xsl[dpi * 64:(dpi + 1) * 64] = xpad[
                :, 16 * dh + dpi:16 * dh + dpi + 17:2,
                16 * hh:16 * hh + 18, :]
        in_maps.append({"xs": xsl, "tcw": tcwb})
    return in_maps


def kernel(x, weight, bias):
    from concourse.bass_utils import run_bass_kernel_spmd

    if "nc" not in _cache:
        _cache["nc"] = _build_nc()
    nc = _cache["nc"]

    x = np.asarray(x, np.float32)
    weight = np.asarray(weight, np.float32)
    bias = np.asarray(bias, np.float32)

    in_maps = _make_in_maps(x, weight, bias)
    res = run_bass_kernel_spmd(nc, in_maps, core_ids=list(range(N_CORES)))

    # every non-conv voxel (even lattice positions, trailing output_padding
    # planes) is exactly bias; fill host-side, add bias to the conv voxels
    # and scatter them into the odd sub-lattice
    full = np.empty((2, 32, 66, 66, 66), np.float32)
    full[:] = bias.reshape(1, 32, 1, 1, 1)
    for core in range(N_CORES):
        n, dh, hh = core // 4, (core // 2) % 2, core % 2
        # [128, 4, 512] -> (qblk, co, j, h, w) -> (co, q=4j+qblk, h, w)
        arr = res.results[core]["out"].astype(np.float32)
        arr = arr.reshape(4, 32, 4, 16, 32).transpose(1, 2, 0, 3, 4)
        arr = arr.reshape(32, 16, 16, 32) + bias.reshape(32, 1, 1, 1)
        full[n, :, 32 * dh + 1:32 * dh + 32:2,
             32 * hh + 1:32 * hh + 32:2, 1:65:2] = arr
    return full
